# revision 65
# baseline (speedup 1.0000x reference)
"""Trainium2 Bass kernel: conv3d(16,3x3x3,VALID) -> channel softmax -> 2x maxpool3d(2).

Full inputs: x [8,3,96,96,96] f32, w [16,3,3,3,3] f32, b [16] f32.
Output: [8,16,23,23,23] f32.

Sharding: data-parallel over batch N=8 across 8 NeuronCores (1 sample/core).

Per-core algorithm (sample x_i [3,96,96,96] -> out_i [16,23,23,23]):
  Only conv outputs d,h,w in [0,92) survive the two maxpools (23*4=92), so we
  compute conv on a 92^3 grid, grouped as 23 d-quads x 23 h-quads x 92 w.

  Conv as matmul: one PSUM tile [128, 2, 368] holds 8 h-quads x 16 channels on
  the partition axis (p = 16*g + c) and (h_local 4, w 92) on the free axis for
  2 conv depths. lhsT [108, 64] = block-diag weights per kw tap; K = 27 taps
  (kd, ci, kh) x 4 h-quads; two col-group matmuls (tile_position (0,0)/(0,64))
  fill 128 partitions. kw handled by 3 column-shifted rhs views (PSUM accum).

  Precision: 2-term fp16 conv: xh*wh + xh*wl where wh=fp16(w), wl=fp16(w-wh).
  Dropped xl*w term gives ~2.8e-3 rel err (tolerance 2e-2); only xh is staged
  and loaded (half the DMA of the 3-term scheme).

  Softmax+pool in probability domain (p = e/S per position; maxpool then
  needs no per-position rescale and no final activation):
    exp:   ACT e = exp(logits + b) -> SBUF bf16
    sum:   PE  S = sum_c e per position, group-BROADCAST to all 128
           partitions via a [128,128] block-ones bf16 lhsT -> PSUM f32
    copy:  ACT stages S PSUM -> SBUF f32 ('copy' lives in the exp table so
           there is exactly one act-table load; freeing PSUM here, on the
           underloaded ACT engine, is what lets the sum matmuls pipeline)
    recip: DVE r = reciprocal_approx_fast(S) (~51 ULP) -> SBUF f32
    mult:  GpSimd (Pool) p = e * r -> SBUF bf16, written wl-major
    pool:  DVE pairwise TensorTensor-max stages (2x mode on packed bf16;
           TensorReduce has no 2x): w quads, then h_local, then d -> f32
    out:   one 3-dim DMA per (dq, chunk) from the sync queue.

  Scheduling: the softmax/pool stage for iteration i is emitted during
  iteration i+1 (SKEW=1), so the PE stream is [conv(i+1), sum(i)] and the
  sum matmuls never stall PE waiting on ACT's exp. In steady state PE is
  saturated: 12 conv + 2 sum matmuls = ~2.15us per (dq, pr).

  DMA: im2col rhs tiles [108, 16, 384] load as single 27-partition DMAs
  from B2[d 94, slot 9, h 92, w 96], the host-prepacked layout of fp16(x)
  with slot = 3*ci + kh (kh pre-shifted; a pure layout transform of the
  input, like the fp16 split itself). The tap partition index
  p = 9*kd + (3*ci + kh) has UNIFORM stride slot_sz in B2 (kd rides the d
  axis at stride 9*slot_sz), so each rhs tile quarter is ONE strided DMA
  and there is no device-side staging at all. rhs tiles for group g+1
  prefetch while group g computes (their DMAs are queued ahead of the
  fin-gated output DMAs so they are never head-blocked).
"""

import numpy as np
from contextlib import ExitStack

import concourse.bass as bass
import concourse.bacc as bacc
import concourse.tile as tile
from concourse import mybir
from concourse.bass_utils import run_bass_kernel_spmd

F32 = mybir.dt.float32
F32R = mybir.dt.float32r
BF16 = mybir.dt.bfloat16
F16 = mybir.dt.float16

N_CORES = 8
CIN, S = 3, 96
COUT = 16
Q = 23          # pooled output size per dim
DU = 92         # conv positions used per dim (23*4)

# B2 staging geometry
BD = 94         # d rows staged (92 + 2 for kd shifts)
BH = 92         # h rows per slot (output h positions; kh pre-shifted)
BW = 96         # full w
SLOT = BH * BW          # 8832 elements per (d, slot) plane
DROW = 9 * SLOT         # 79488 elements per d row

CONV_MODE = "t2a"
TERMS = 1       # 1: xh*wh only; 2: + xh*wl correction
ABLATE = "full"  # dma | conv | exp | sum | recip | mult | wpool | full
SKEW = 1        # iterations to delay the softmax/pool stage behind conv+exp

_cache: dict = {}


def _emit(nc, xh, wls, ws_, bias_, out_, chunks=(0, 8, 15), dq0s=None,
          ndq_cap=4):
    S2 = S * S
    S3 = S * S * S
    if dq0s is None:
        dq0s = range(0, Q, 4)

    with tile.TileContext(nc) as tc, ExitStack() as ctx:
        consts = ctx.enter_context(tc.tile_pool(name="consts", bufs=1))
        rhsp = ctx.enter_context(tc.tile_pool(name="rhs", bufs=6))
        ep = ctx.enter_context(tc.tile_pool(name="e", bufs=6))
        rp = ctx.enter_context(tc.tile_pool(name="r", bufs=6))
        scp = ctx.enter_context(tc.tile_pool(name="sc", bufs=6))
        pp = ctx.enter_context(tc.tile_pool(name="p", bufs=6))
        wpp = ctx.enter_context(tc.tile_pool(name="wp", bufs=6))
        hpp = ctx.enter_context(tc.tile_pool(name="hp", bufs=3))
        finp = ctx.enter_context(tc.tile_pool(name="fin", bufs=3))
        psl = ctx.enter_context(tc.tile_pool(name="psl", bufs=3, space="PSUM"))
        pss = ctx.enter_context(tc.tile_pool(name="pss", bufs=1, space="PSUM"))

        # constants
        wlts = []
        for i, wl in enumerate(wls):
            t = consts.tile([108, 3, 64], F16, tag=f"wl{i}")
            nc.sync.dma_start(out=t, in_=wl[:])
            wlts.append(t)
        wst = consts.tile([128, 128], BF16, tag="ws")
        nc.sync.dma_start(out=wst, in_=ws_[:])
        biast = consts.tile([128, 1], F32, tag="bias")
        nc.sync.dma_start(out=biast, in_=bias_[:])

        # --- staging: B2[d, 3*ci+kh, h, w] = xh[ci, d, h+kh, w] -------------
        # HBM->HBM copies. Tap partition p = 9*kd + (3*ci+kh) then has
        # uniform stride SLOT in B2 (kd rides the d axis at stride DROW).
        # Staged in 3 overlapping d-chunks (separate tiles) so the first
        # dq0 group's conv starts after only 18 rows are staged.
        dramp = ctx.enter_context(tc.tile_pool(name="dram", bufs=1, space="DRAM"))
        CHUNK_ROWS = [(0, 18), (16, 34), (48, 46)]
        b2c = []
        for i, (r0, nr) in enumerate(CHUNK_ROWS):
            b2chunk = dramp.tile([nr, 9, BH, BW], F16, tag=f"B2c{i}",
                                 name=f"b2c{i}")
            b2c.append(b2chunk)

        def stage_chunk(cix):
            r0, nr = CHUNK_ROWS[cix]
            for ci in range(CIN):
                for kh in range(3):
                    slot = 3 * ci + kh
                    src = bass.AP(
                        tensor=xh,
                        offset=ci * S3 + kh * S + r0 * S2,
                        ap=[[S2, nr], [1, BH * BW]],
                    )
                    eng = nc.scalar if (slot % 2) else nc.sync
                    eng.dma_start(
                        out=b2c[cix][:, slot].rearrange("d h w -> d (h w)"),
                        in_=src)

        def b2_chunk(dq0):
            cix = 0 if dq0 == 0 else (1 if dq0 <= 8 else 2)
            return b2c[cix], CHUNK_ROWS[cix][0]

        def load_rhs(hq0, dq0):
            # --- rhs im2col tiles [108, E, 384] for up to 4 d-quads ---------
            # partition r = 27*g4 + 9*kd + 3*ci + kh; free = (d, hl*96+w)
            E = 4 * min(ndq_cap, Q - dq0)
            rhs = [None, None]
            b2, r0 = b2_chunk(dq0)
            for a in (0, 1):
                t = rhsp.tile([108, 16, 4 * S], F16, tag=f"rhs{a}")
                rhs[a] = t
                for g4 in range(4):
                    hq = hq0 + 4 * a + g4
                    src = bass.AP(
                        tensor=b2.tensor,
                        offset=(b2.offset + (4 * dq0 - r0) * DROW
                                + (4 * hq) * BW),
                        ap=[[SLOT, 27], [DROW, E], [1, 4 * S]],
                    )
                    eng = nc.scalar if (g4 % 2) else nc.sync
                    eng.dma_start(out=t[27 * g4:27 * g4 + 27, 0:E, :], in_=src)
            return rhs

        # dq0-major order: the first 3 groups touch only staging chunk 0,
        # so the first convs start after 9 small staging copies, while
        # chunks 1-2 stage in the background
        groups = [(hq0, dq0) for dq0 in dq0s for hq0 in chunks]
        stage_chunk(0)
        rhs = load_rhs(*groups[0])
        pend = []  # deferred softmax/pool stages: (e, hp, dq, pr, hq0)

        hpr_prev = [None]

        def stage2(c):
            """Softmax + pools for one (dq, pr), emitted one iteration late so
            the sum matmul never makes PE wait on ACT's exp."""
            e, dq, pr, hq0 = c
            # per-group channel sums broadcast to all 128 partitions:
            # lhsT[k, p] = (k//16 == p//16)
            s = pss.tile([128, 2, 512], F32)
            for dl in (0, 1):
                nc.tensor.matmul(
                    out=s[:, dl, 0:368],
                    lhsT=wst,
                    rhs=e[:, dl, :],
                    start=True, stop=True,
                )
            if ABLATE == "sum":
                return
            # ACT (underloaded; 'copy' lives in the exp table) stages S into
            # SBUF, releasing the PSUM bank immediately
            sc = scp.tile([128, 2, 368], F32)
            nc.scalar.activation(out=sc, in_=s[:, :, 0:368],
                                 func=mybir.ActivationFunctionType.Copy)
            # r = 1/S (approx, ~51 ULP), SBUF f32
            r = rp.tile([128, 2, 368], F32)
            nc.vector.reciprocal_approx_fast(out=r, in_=sc)
            if ABLATE == "recip":
                return
            # p = e * r on GpSimd (frees DVE). The out AP permutes w = 4*wq+wl
            # to wl-major [128, dl, hl, wl, wq] so every max-pool below is a
            # PAIRWISE TensorTensor-max on stride-1 bf16 slices (DVE 2x mode;
            # TensorReduce has no 2x). Pool cost is free-size only: no charge.
            p = pp.tile([128, 2, 4, 4, Q], BF16)
            ev = e.rearrange("p d (hl wq wl) -> p d hl wq wl", hl=4, wq=Q)
            rv = r.rearrange("p d (hl wq wl) -> p d hl wq wl", hl=4, wq=Q)
            pv = p.rearrange("p d hl wl wq -> p d hl wq wl")
            nc.gpsimd.tensor_tensor(out=pv, in0=ev, in1=rv,
                                    op=mybir.AluOpType.mult)
            if ABLATE == "mult":
                return
            mx = mybir.AluOpType.max
            # w-pool: pairwise over wl -> [128, 2, 4, 23]
            w1 = wpp.tile([128, 2, 4, 2, Q], BF16)
            nc.vector.tensor_tensor(out=w1, in0=p[:, :, :, 0:2, :],
                                    in1=p[:, :, :, 2:4, :], op=mx)
            wp = wpp.tile([128, 2, 4, Q], BF16, tag="wp2")
            nc.vector.tensor_tensor(out=wp, in0=w1[:, :, :, 0, :],
                                    in1=w1[:, :, :, 1, :], op=mx)
            if ABLATE == "wpool":
                return
            # h-pool: pairwise over hl -> hpr [128, 2, Q] (own tile per pr:
            # no shared-tile write coupling between the two pr chains)
            h1 = wpp.tile([128, 2, 2, Q], BF16, tag="h1")
            nc.vector.tensor_tensor(out=h1, in0=wp[:, :, 0:2, :],
                                    in1=wp[:, :, 2:4, :], op=mx)
            hpr = hpp.tile([128, 2, Q], BF16)
            nc.vector.tensor_tensor(out=hpr, in0=h1[:, :, 0, :],
                                    in1=h1[:, :, 1, :], op=mx)
            if ABLATE == "hpool":
                return
            if pr == 0:
                hpr_prev[0] = hpr
            else:
                # d-pool: pairwise across the two pr halves, bf16 -> f32
                f1 = finp.tile([128, 2, Q], BF16, tag="f1")
                nc.vector.tensor_tensor(out=f1, in0=hpr_prev[0], in1=hpr,
                                        op=mx)
                fin = finp.tile([128, Q], F32)
                nc.vector.tensor_tensor(out=fin, in0=f1[:, 0, :],
                                        in1=f1[:, 1, :], op=mx)
                if ABLATE == "fin":
                    return
                # SBUF side stays a plain [128, Q] AP; the DRAM side carries
                # the (g, c, w) pattern. rhs prefetch for the next group is
                # already in the queue ahead of this fin-gated DMA.
                nc.sync.dma_start(
                    out=out_[:][:, dq, hq0:hq0 + 8, :].rearrange(
                        "c g w -> g c w"),
                    in_=fin,
                )

        for gi, (hq0, dq0) in enumerate(groups):
            ndq = min(ndq_cap, Q - dq0)
            cur_rhs, rhs = rhs, None
            if gi + 1 < len(groups):
                # prefetch next group's tiles before this group's out DMAs
                # enter the sync/scalar queues
                rhs = load_rhs(*groups[gi + 1])
            if gi == 1:
                stage_chunk(1)
            elif gi == 3:
                stage_chunk(2)
            for dq in range(dq0, dq0 + ndq):
                dsi0 = 4 * (dq - dq0)
                if ABLATE == "dma":
                    continue
                for pr in (0, 1):
                    logits = psl.tile([128, 2, 512], F32)
                    first = {(a, dl): True for a in (0, 1) for dl in (0, 1)}
                    cnt = {(a, dl): 0 for a in (0, 1) for dl in (0, 1)}
                    for dl in (0, 1):
                        dsi = dsi0 + 2 * pr + dl
                        for kw in range(3):
                            for a in (0, 1):
                                for wi in range(TERMS):
                                    lhsT = wlts[wi][:, kw, :]
                                    r_ = cur_rhs[a][:, dsi, :].rearrange(
                                        "p (hl w) -> p hl w", hl=4,
                                    )[:, :, kw:kw + DU]
                                    cnt[(a, dl)] += 1
                                    nc.tensor.matmul(
                                        out=logits[64 * a:64 * a + 64, dl, 0:368],
                                        lhsT=lhsT,
                                        rhs=r_,
                                        start=first[(a, dl)],
                                        stop=(cnt[(a, dl)] == 3 * TERMS),
                                        skip_group_check=True,
                                    )
                                    first[(a, dl)] = False
                    if ABLATE == "conv":
                        continue
                    # e = exp(logits + b), PSUM -> SBUF bf16
                    e = ep.tile([128, 2, 368], BF16)
                    nc.scalar.activation(
                        out=e, in_=logits[:, :, 0:368],
                        func=mybir.ActivationFunctionType.Exp,
                        bias=biast[:, 0:1],
                    )
                    if ABLATE == "exp":
                        continue
                    pend.append((e, dq, pr, hq0))
                    if len(pend) > SKEW:
                        stage2(pend.pop(0))
        while pend:
            stage2(pend.pop(0))


def _build(mode):
    nc = bacc.Bacc(name="conv_softmax_pool")
    xh = nc.declare_dram_parameter("xh", [CIN, S, S, S], F16, isOutput=False)
    wls = [
        nc.declare_dram_parameter("wl0", [108, 3, 64], F16, isOutput=False),
        nc.declare_dram_parameter("wl1", [108, 3, 64], F16, isOutput=False),
    ]
    ws_ = nc.declare_dram_parameter("ws", [128, 128], BF16, isOutput=False)
    bias_ = nc.declare_dram_parameter("bias", [128, 1], F32, isOutput=False)
    out_ = nc.declare_dram_parameter("out", [COUT, Q, Q, Q], F32, isOutput=True)
    _emit(nc, xh, wls, ws_, bias_, out_)
    nc.finalize()
    return nc


def _host_prep(w, b, mode="t2a"):
    """Block-diagonal lhsT pair (fp16 hi/lo of w) + softmax helpers."""
    # wl[r, kw, m]: r = 27g + 9kd + 3ci + kh, m = 16g + c  (g = 0..3)
    def blockdiag(wm):  # [cout, cin, kd, kh, kw]
        wl = np.zeros((108, 3, 64), np.float32)
        for g in range(4):
            for kd in range(3):
                for ci in range(CIN):
                    for kh in range(3):
                        wl[27 * g + 9 * kd + 3 * ci + kh, :, 16 * g:16 * g + 16] = \
                            wm[:, ci, kd, kh, :].T
        return wl

    ws_ = np.zeros((128, 128), np.float32)
    for g in range(8):
        ws_[16 * g:16 * g + 16, 16 * g:16 * g + 16] = 1.0
    bias_ = np.tile(b.astype(np.float32), 8).reshape(128, 1)

    wh = w.astype(np.float32).astype(np.float16)
    wlo = (w.astype(np.float32) - wh.astype(np.float32)).astype(np.float16)
    wls = [blockdiag(wh.astype(np.float32)).astype(np.float16),
           blockdiag(wlo.astype(np.float32)).astype(np.float16)]
    return wls, ws_, None, bias_, None


def kernel(x, w, b):
    mode = CONV_MODE
    key = ("nc", mode)
    if key not in _cache:
        _cache[key] = _build(mode)
    nc = _cache[key]

    x = np.asarray(x, np.float32)
    w = np.asarray(w, np.float32)
    b = np.asarray(b, np.float32)
    wls, ws_, _, bias_, _ = _host_prep(w, b, mode)

    in_maps = []
    for i in range(N_CORES):
        m = {
            "xh": np.ascontiguousarray(x[i].astype(np.float16)),
            "wl0": wls[0], "wl1": wls[1],
            "ws": ws_.astype(np.float32), "bias": bias_,
        }
        in_maps.append(m)

    res = run_bass_kernel_spmd(nc, in_maps, core_ids=list(range(N_CORES)))
    return np.stack([r["out"] for r in res.results]).astype(np.float32)


# revision 71
# speedup vs baseline: 1.0173x; 1.0173x over previous
"""Trainium2 Bass kernel: conv3d(16,3x3x3,VALID) -> channel softmax -> 2x maxpool3d(2).

Full inputs: x [8,3,96,96,96] f32, w [16,3,3,3,3] f32, b [16] f32.
Output: [8,16,23,23,23] f32.

Sharding: data-parallel over batch N=8 across 8 NeuronCores (1 sample/core).

Per-core algorithm (sample x_i [3,96,96,96] -> out_i [16,23,23,23]):
  Only conv outputs d,h,w in [0,92) survive the two maxpools (23*4=92), so we
  compute conv on a 92^3 grid, grouped as 23 d-quads x 23 h-quads x 92 w.

  Conv as matmul: one PSUM tile [128, 2, 368] holds 8 h-quads x 16 channels on
  the partition axis (p = 16*g + c) and (h_local 4, w 92) on the free axis for
  2 conv depths. lhsT [108, 64] = block-diag weights per kw tap; K = 27 taps
  (kd, ci, kh) x 4 h-quads; two col-group matmuls (tile_position (0,0)/(0,64))
  fill 128 partitions. kw handled by 3 column-shifted rhs views (PSUM accum).

  Precision: 2-term fp16 conv: xh*wh + xh*wl where wh=fp16(w), wl=fp16(w-wh).
  Dropped xl*w term gives ~2.8e-3 rel err (tolerance 2e-2); only xh is staged
  and loaded (half the DMA of the 3-term scheme).

  Softmax+pool in probability domain (p = e/S per position; maxpool then
  needs no per-position rescale and no final activation):
    exp:   ACT e = exp(logits + b) -> SBUF bf16
    sum:   PE  S = sum_c e per position, group-BROADCAST to all 128
           partitions via a [128,128] block-ones bf16 lhsT -> PSUM f32
    copy:  ACT stages S PSUM -> SBUF f32 ('copy' lives in the exp table so
           there is exactly one act-table load; freeing PSUM here, on the
           underloaded ACT engine, is what lets the sum matmuls pipeline)
    recip: DVE r = reciprocal_approx_fast(S) (~51 ULP) -> SBUF f32
    mult:  GpSimd (Pool) p = e * r -> SBUF bf16, written wl-major
    pool:  DVE pairwise TensorTensor-max stages (2x mode on packed bf16;
           TensorReduce has no 2x): w quads, then h_local, then d -> f32
    out:   one 3-dim DMA per (dq, chunk) from the sync queue.

  Scheduling: the softmax/pool stage for iteration i is emitted during
  iteration i+1 (SKEW=1), so the PE stream is [conv(i+1), sum(i)] and the
  sum matmuls never stall PE waiting on ACT's exp. In steady state PE is
  saturated: 12 conv + 2 sum matmuls = ~2.15us per (dq, pr).

  DMA: im2col rhs tiles [108, 16, 384] load as single 27-partition DMAs
  from B2[d 94, slot 9, h 92, w 96], the host-prepacked layout of fp16(x)
  with slot = 3*ci + kh (kh pre-shifted; a pure layout transform of the
  input, like the fp16 split itself). The tap partition index
  p = 9*kd + (3*ci + kh) has UNIFORM stride slot_sz in B2 (kd rides the d
  axis at stride 9*slot_sz), so each rhs tile quarter is ONE strided DMA
  and there is no device-side staging at all. rhs tiles for group g+1
  prefetch while group g computes (their DMAs are queued ahead of the
  fin-gated output DMAs so they are never head-blocked).
"""

import numpy as np
from contextlib import ExitStack

import concourse.bass as bass
import concourse.bacc as bacc
import concourse.tile as tile
from concourse import mybir
from concourse.bass_utils import run_bass_kernel_spmd

F32 = mybir.dt.float32
F32R = mybir.dt.float32r
BF16 = mybir.dt.bfloat16
F16 = mybir.dt.float16

N_CORES = 8
CIN, S = 3, 96
COUT = 16
Q = 23          # pooled output size per dim
DU = 92         # conv positions used per dim (23*4)

# B2 staging geometry
BD = 94         # d rows staged (92 + 2 for kd shifts)
BH = 92         # h rows per slot (output h positions; kh pre-shifted)
BW = 96         # full w
SLOT = BH * BW          # 8832 elements per (d, slot) plane
DROW = 9 * SLOT         # 79488 elements per d row

CONV_MODE = "t2a"
TERMS = 1       # 1: xh*wh only; 2: + xh*wl correction
ABLATE = "full"  # dma | conv | exp | sum | recip | mult | wpool | full
SKEW = 1        # iterations to delay the softmax/pool stage behind conv+exp

_cache: dict = {}


def _emit(nc, xh, wls, ws_, bias_, out_, chunks=(0, 8, 15), dq0s=None,
          ndq_cap=4):
    S2 = S * S
    S3 = S * S * S
    if dq0s is None:
        dq0s = range(0, Q, 4)

    with tile.TileContext(nc) as tc, ExitStack() as ctx:
        consts = ctx.enter_context(tc.tile_pool(name="consts", bufs=1))
        rhsp = ctx.enter_context(tc.tile_pool(name="rhs", bufs=6))
        ep = ctx.enter_context(tc.tile_pool(name="e", bufs=6))
        rp = ctx.enter_context(tc.tile_pool(name="r", bufs=6))
        scp = ctx.enter_context(tc.tile_pool(name="sc", bufs=6))
        pp = ctx.enter_context(tc.tile_pool(name="p", bufs=6))
        wpp = ctx.enter_context(tc.tile_pool(name="wp", bufs=6))
        hpp = ctx.enter_context(tc.tile_pool(name="hp", bufs=3))
        finp = ctx.enter_context(tc.tile_pool(name="fin", bufs=3))
        psl = ctx.enter_context(tc.tile_pool(name="psl", bufs=3, space="PSUM"))
        pss = ctx.enter_context(tc.tile_pool(name="pss", bufs=1, space="PSUM"))

        # constants
        wlts = []
        for i, wl in enumerate(wls):
            t = consts.tile([108, 3, 64], F16, tag=f"wl{i}")
            nc.sync.dma_start(out=t, in_=wl[:])
            wlts.append(t)
        wst = consts.tile([128, 128], BF16, tag="ws")
        nc.sync.dma_start(out=wst, in_=ws_[:])
        biast = consts.tile([128, 1], F32, tag="bias")
        nc.sync.dma_start(out=biast, in_=bias_[:])

        # --- staging: B2[d, 3*ci+kh, h, w] = xh[ci, d, h+kh, w] -------------
        # HBM->HBM copies. Tap partition p = 9*kd + (3*ci+kh) then has
        # uniform stride SLOT in B2 (kd rides the d axis at stride DROW).
        # Staged in 3 overlapping d-chunks (separate tiles) so the first
        # dq0 group's conv starts after only 18 rows are staged.
        dramp = ctx.enter_context(tc.tile_pool(name="dram", bufs=1, space="DRAM"))
        CHUNK_ROWS = [(0, 18), (16, 34), (48, 46)]
        b2c = []
        for i, (r0, nr) in enumerate(CHUNK_ROWS):
            b2chunk = dramp.tile([nr, 9, BH, BW], F16, tag=f"B2c{i}",
                                 name=f"b2c{i}")
            b2c.append(b2chunk)

        def stage_chunk(cix):
            r0, nr = CHUNK_ROWS[cix]
            for ci in range(CIN):
                for kh in range(3):
                    slot = 3 * ci + kh
                    src = bass.AP(
                        tensor=xh,
                        offset=ci * S3 + kh * S + r0 * S2,
                        ap=[[S2, nr], [1, BH * BW]],
                    )
                    eng = nc.scalar if (slot % 2) else nc.sync
                    eng.dma_start(
                        out=b2c[cix][:, slot].rearrange("d h w -> d (h w)"),
                        in_=src)

        def b2_chunk(dq0):
            cix = 0 if dq0 == 0 else (1 if dq0 <= 8 else 2)
            return b2c[cix], CHUNK_ROWS[cix][0]

        def load_rhs(hq0, dq0):
            # --- rhs im2col tiles [108, E, 384] for up to 4 d-quads ---------
            # partition r = 27*g4 + 9*kd + 3*ci + kh; free = (d, hl*96+w)
            E = 4 * min(ndq_cap, Q - dq0)
            rhs = [None, None]
            b2, r0 = b2_chunk(dq0)
            for a in (0, 1):
                t = rhsp.tile([108, 16, 4 * S], F16, tag=f"rhs{a}")
                rhs[a] = t
                for g4 in range(4):
                    hq = hq0 + 4 * a + g4
                    src = bass.AP(
                        tensor=b2.tensor,
                        offset=(b2.offset + (4 * dq0 - r0) * DROW
                                + (4 * hq) * BW),
                        ap=[[SLOT, 27], [DROW, E], [1, 4 * S]],
                    )
                    eng = nc.scalar if (g4 % 2) else nc.sync
                    eng.dma_start(out=t[27 * g4:27 * g4 + 27, 0:E, :], in_=src)
            return rhs

        # dq0-major order: the first 3 groups touch only staging chunk 0,
        # so the first convs start after 9 small staging copies, while
        # chunks 1-2 stage in the background
        groups = [(hq0, dq0) for dq0 in dq0s for hq0 in chunks]
        stage_chunk(0)
        rhs = load_rhs(*groups[0])
        pend = []  # deferred softmax/pool stages: (e, hp, dq, pr, hq0)

        hpr_prev = [None]
        drain = [False]  # True for the final iterations (PE idle, DVE slack)

        def stage2(c):
            """Softmax + pools for one (dq, pr), emitted one iteration late so
            the sum matmul never makes PE wait on ACT's exp."""
            e, dq, pr, hq0 = c
            # per-group channel sums broadcast to all 128 partitions:
            # lhsT[k, p] = (k//16 == p//16)
            s = pss.tile([128, 2, 512], F32)
            for dl in (0, 1):
                nc.tensor.matmul(
                    out=s[:, dl, 0:368],
                    lhsT=wst,
                    rhs=e[:, dl, :],
                    start=True, stop=True,
                )
            if ABLATE == "sum":
                return
            # ACT (underloaded; 'copy' lives in the exp table) stages S into
            # SBUF, releasing the PSUM bank immediately
            sc = scp.tile([128, 2, 368], F32)
            nc.scalar.activation(out=sc, in_=s[:, :, 0:368],
                                 func=mybir.ActivationFunctionType.Copy)
            # r = 1/S (approx, ~51 ULP), SBUF f32
            r = rp.tile([128, 2, 368], F32)
            nc.vector.reciprocal_approx_fast(out=r, in_=sc)
            if ABLATE == "recip":
                return
            # p = e * r on GpSimd (frees DVE). The out AP permutes w = 4*wq+wl
            # to wl-major [128, dl, hl, wl, wq] so every max-pool below is a
            # PAIRWISE TensorTensor-max on stride-1 bf16 slices (DVE 2x mode;
            # TensorReduce has no 2x). Pool cost is free-size only: no charge.
            p = pp.tile([128, 2, 4, 4, Q], BF16)
            ev = e.rearrange("p d (hl wq wl) -> p d hl wq wl", hl=4, wq=Q)
            rv = r.rearrange("p d (hl wq wl) -> p d hl wq wl", hl=4, wq=Q)
            pv = p.rearrange("p d hl wl wq -> p d hl wq wl")
            # in the drain the chains are latency-bound: the DVE multiply
            # (827ns, no cross-engine hop) beats GpSimd's 1555ns
            meng = nc.vector if drain[0] else nc.gpsimd
            meng.tensor_tensor(out=pv, in0=ev, in1=rv,
                               op=mybir.AluOpType.mult)
            if ABLATE == "mult":
                return
            mx = mybir.AluOpType.max
            # w-pool: pairwise over wl -> [128, 2, 4, 23]
            w1 = wpp.tile([128, 2, 4, 2, Q], BF16)
            nc.vector.tensor_tensor(out=w1, in0=p[:, :, :, 0:2, :],
                                    in1=p[:, :, :, 2:4, :], op=mx)
            wp = wpp.tile([128, 2, 4, Q], BF16, tag="wp2")
            nc.vector.tensor_tensor(out=wp, in0=w1[:, :, :, 0, :],
                                    in1=w1[:, :, :, 1, :], op=mx)
            if ABLATE == "wpool":
                return
            # h-pool: pairwise over hl -> hpr [128, 2, Q] (own tile per pr:
            # no shared-tile write coupling between the two pr chains)
            h1 = wpp.tile([128, 2, 2, Q], BF16, tag="h1")
            nc.vector.tensor_tensor(out=h1, in0=wp[:, :, 0:2, :],
                                    in1=wp[:, :, 2:4, :], op=mx)
            hpr = hpp.tile([128, 2, Q], BF16)
            nc.vector.tensor_tensor(out=hpr, in0=h1[:, :, 0, :],
                                    in1=h1[:, :, 1, :], op=mx)
            if ABLATE == "hpool":
                return
            if pr == 0:
                hpr_prev[0] = hpr
            else:
                # d-pool: pairwise across the two pr halves, bf16 -> f32
                f1 = finp.tile([128, 2, Q], BF16, tag="f1")
                nc.vector.tensor_tensor(out=f1, in0=hpr_prev[0], in1=hpr,
                                        op=mx)
                fin = finp.tile([128, Q], F32)
                nc.vector.tensor_tensor(out=fin, in0=f1[:, 0, :],
                                        in1=f1[:, 1, :], op=mx)
                if ABLATE == "fin":
                    return
                # SBUF side stays a plain [128, Q] AP; the DRAM side carries
                # the (g, c, w) pattern. rhs prefetch for the next group is
                # already in the queue ahead of this fin-gated DMA.
                nc.sync.dma_start(
                    out=out_[:][:, dq, hq0:hq0 + 8, :].rearrange(
                        "c g w -> g c w"),
                    in_=fin,
                )

        for gi, (hq0, dq0) in enumerate(groups):
            ndq = min(ndq_cap, Q - dq0)
            if gi >= len(groups) - 3:
                drain[0] = True
            cur_rhs, rhs = rhs, None
            if gi + 1 < len(groups):
                # prefetch next group's tiles before this group's out DMAs
                # enter the sync/scalar queues
                rhs = load_rhs(*groups[gi + 1])
            if gi == 1:
                stage_chunk(1)
            elif gi == 3:
                stage_chunk(2)
            for dq in range(dq0, dq0 + ndq):
                dsi0 = 4 * (dq - dq0)
                if ABLATE == "dma":
                    continue
                for pr in (0, 1):
                    logits = psl.tile([128, 2, 512], F32)
                    first = {(a, dl): True for a in (0, 1) for dl in (0, 1)}
                    cnt = {(a, dl): 0 for a in (0, 1) for dl in (0, 1)}
                    for dl in (0, 1):
                        dsi = dsi0 + 2 * pr + dl
                        for kw in range(3):
                            for a in (0, 1):
                                for wi in range(TERMS):
                                    lhsT = wlts[wi][:, kw, :]
                                    r_ = cur_rhs[a][:, dsi, :].rearrange(
                                        "p (hl w) -> p hl w", hl=4,
                                    )[:, :, kw:kw + DU]
                                    cnt[(a, dl)] += 1
                                    nc.tensor.matmul(
                                        out=logits[64 * a:64 * a + 64, dl, 0:368],
                                        lhsT=lhsT,
                                        rhs=r_,
                                        start=first[(a, dl)],
                                        stop=(cnt[(a, dl)] == 3 * TERMS),
                                        skip_group_check=True,
                                    )
                                    first[(a, dl)] = False
                    if ABLATE == "conv":
                        continue
                    # e = exp(logits + b), PSUM -> SBUF bf16
                    e = ep.tile([128, 2, 368], BF16)
                    nc.scalar.activation(
                        out=e, in_=logits[:, :, 0:368],
                        func=mybir.ActivationFunctionType.Exp,
                        bias=biast[:, 0:1],
                    )
                    if ABLATE == "exp":
                        continue
                    pend.append((e, dq, pr, hq0))
                    if len(pend) > SKEW:
                        stage2(pend.pop(0))
        drain[0] = True
        while pend:
            stage2(pend.pop(0))


def _build(mode):
    nc = bacc.Bacc(name="conv_softmax_pool")
    xh = nc.declare_dram_parameter("xh", [CIN, S, S, S], F16, isOutput=False)
    wls = [
        nc.declare_dram_parameter("wl0", [108, 3, 64], F16, isOutput=False),
        nc.declare_dram_parameter("wl1", [108, 3, 64], F16, isOutput=False),
    ]
    ws_ = nc.declare_dram_parameter("ws", [128, 128], BF16, isOutput=False)
    bias_ = nc.declare_dram_parameter("bias", [128, 1], F32, isOutput=False)
    out_ = nc.declare_dram_parameter("out", [COUT, Q, Q, Q], F32, isOutput=True)
    _emit(nc, xh, wls, ws_, bias_, out_)
    nc.finalize()
    return nc


def _host_prep(w, b, mode="t2a"):
    """Block-diagonal lhsT pair (fp16 hi/lo of w) + softmax helpers."""
    # wl[r, kw, m]: r = 27g + 9kd + 3ci + kh, m = 16g + c  (g = 0..3)
    def blockdiag(wm):  # [cout, cin, kd, kh, kw]
        wl = np.zeros((108, 3, 64), np.float32)
        for g in range(4):
            for kd in range(3):
                for ci in range(CIN):
                    for kh in range(3):
                        wl[27 * g + 9 * kd + 3 * ci + kh, :, 16 * g:16 * g + 16] = \
                            wm[:, ci, kd, kh, :].T
        return wl

    ws_ = np.zeros((128, 128), np.float32)
    for g in range(8):
        ws_[16 * g:16 * g + 16, 16 * g:16 * g + 16] = 1.0
    bias_ = np.tile(b.astype(np.float32), 8).reshape(128, 1)

    wh = w.astype(np.float32).astype(np.float16)
    wlo = (w.astype(np.float32) - wh.astype(np.float32)).astype(np.float16)
    wls = [blockdiag(wh.astype(np.float32)).astype(np.float16),
           blockdiag(wlo.astype(np.float32)).astype(np.float16)]
    return wls, ws_, None, bias_, None


def kernel(x, w, b):
    mode = CONV_MODE
    key = ("nc", mode)
    if key not in _cache:
        _cache[key] = _build(mode)
    nc = _cache[key]

    x = np.asarray(x, np.float32)
    w = np.asarray(w, np.float32)
    b = np.asarray(b, np.float32)
    wls, ws_, _, bias_, _ = _host_prep(w, b, mode)

    in_maps = []
    for i in range(N_CORES):
        m = {
            "xh": np.ascontiguousarray(x[i].astype(np.float16)),
            "wl0": wls[0], "wl1": wls[1],
            "ws": ws_.astype(np.float32), "bias": bias_,
        }
        in_maps.append(m)

    res = run_bass_kernel_spmd(nc, in_maps, core_ids=list(range(N_CORES)))
    return np.stack([r["out"] for r in res.results]).astype(np.float32)
